# revision 4
# baseline (speedup 1.0000x reference)
"""Batch graph attention (GAT-style) Trainium2 kernel.

Problem: B=8, N=2048, F=64, FH=64, H=4.
  feats = X @ W[h]                         [B,H,N,FH]
  scores[n,m] = leaky_relu(s_self[n] + s_neigh[m], 0.2)
  P = softmax(scores + (1-A)*NEG_BIG, axis=m)
  out = relu(concat_h(P @ feats + b))

Sharding: batch b -> core b (8 cores, data parallel).

Per-core algorithm (all in "transposed" orientation so the PE can reduce
over the neighbor index m, which must sit on SBUF partitions):

  exp(leaky(x)) == max(e^x, e^{0.2x})  (slope<1), and each branch factors
  rank-1 over (n, m).  Dropping the per-column factor e^{s_self[n]}
  (softmax columns are scale invariant) leaves

      Phat[m,n] = A^T[m,n] * max(e1[m], e2[m] * g[n])

  with e1=exp(s_neigh), e2=exp(0.2*s_neigh), g=exp(-0.8*s_self).
  Aggregation + denominators come from one PE matmul stream per m-tile:

      acc[o,n] += G[m,o]^T Phat[m,n],   G = [feats + b | 1]

  and out[n, h*64+o] = relu(acc[o,n] / acc[64,n]) is produced transposed
  ([H,FH,N] per core) and untransposed on the host during unsharding.

  A^T is produced on-chip by bitcasting the fp32 A (values 0.0/1.0) to
  bf16 pairs [0x0000 | bf16(A)], xbar-DMA-transposing 128-column chunks
  (interleaved zero rows), and compacting odd partitions with two
  constant permutation matmuls on the PE.
"""

import numpy as np

B, N, F, FH, H = 8, 2048, 64, 64, 4
P = 128           # SBUF partitions
NT = N // P       # 16 m-tiles
C = 512           # matmul moving-operand chunk
NCH = N // C      # 4 chunks
GW = 66           # G row stride (64 feats + 1 ones + 1 pad)

_CACHE = {}


def _build():
    import concourse.bacc as bacc
    import concourse.tile as tile
    import concourse.mybir as mybir
    from concourse.mybir import AluOpType as op, ActivationFunctionType as act

    f32 = mybir.dt.float32
    bf16 = mybir.dt.bfloat16
    fp16 = mybir.dt.float16
    i32 = mybir.dt.int32

    nc = bacc.Bacc(
        "TRN2",
        target_bir_lowering=False,
        debug=False,
        enable_asserts=False,
        num_devices=8,
    )

    A_d = nc.dram_tensor("A", [N, N], f32, kind="ExternalInput").ap()
    X_d = nc.dram_tensor("X", [N, F], f32, kind="ExternalInput").ap()
    W_d = nc.dram_tensor("W", [H, F, FH], f32, kind="ExternalInput").ap()
    b_d = nc.dram_tensor("b", [H, FH], f32, kind="ExternalInput").ap()
    as_d = nc.dram_tensor("a_self", [H, FH], f32, kind="ExternalInput").ap()
    an_d = nc.dram_tensor("a_neigh", [H, FH], f32, kind="ExternalInput").ap()
    OUT_d = nc.dram_tensor("OUT", [H, FH, N], f32, kind="ExternalOutput").ap()

    with tile.TileContext(nc) as tc:
        with (
            tc.tile_pool(name="const", bufs=1) as const,
            tc.tile_pool(name="big", bufs=1) as big,
            tc.tile_pool(name="stream", bufs=3) as stream,
            tc.tile_pool(name="head", bufs=2) as head,
            tc.tile_pool(name="outp", bufs=3) as outp,
            tc.tile_pool(name="ps", bufs=2, space="PSUM") as ps,
        ):
            # ---- constants --------------------------------------------
            iota_i = const.tile([P, P], i32)
            nc.gpsimd.iota(iota_i[:], pattern=[[1, P]], base=0, channel_multiplier=0)
            pidx_i = const.tile([P, 1], i32)
            nc.gpsimd.iota(pidx_i[:], pattern=[[0, 1]], base=0, channel_multiplier=1)
            iota_f = const.tile([P, P], f32)
            nc.vector.tensor_copy(iota_f[:], iota_i[:])
            pidx_f = const.tile([P, 1], f32)
            nc.vector.tensor_copy(pidx_f[:], pidx_i[:])
            ident = const.tile([P, P], f32)
            nc.vector.tensor_scalar(ident[:], iota_f[:], pidx_f[:], None, op.is_equal)
            pm1 = const.tile([P, 1], f32)
            nc.vector.tensor_scalar(pm1[:], pidx_f[:], 1.0, None, op.subtract)
            pp127 = const.tile([P, 1], f32)
            nc.vector.tensor_scalar(pp127[:], pidx_f[:], 127.0, None, op.add)
            # perm_a[p,q]=1 iff p==2q+1 ; perm_b[p,q]=1 iff p==2q-127
            perm_a = const.tile([P, P], bf16)
            nc.vector.tensor_scalar(
                perm_a[:], iota_f[:], 2.0, pm1[:], op.mult, op.is_equal
            )
            perm_b = const.tile([P, P], bf16)
            nc.vector.tensor_scalar(
                perm_b[:], iota_f[:], 2.0, pp127[:], op.mult, op.is_equal
            )
            ones_b = const.tile([1, P], bf16)
            ones_h = const.tile([1, P], fp16)
            nc.vector.memset(ones_h[:], 1.0)
            ones_f = const.tile([1, P], f32)
            nc.vector.memset(ones_f[:], 1.0)
            nc.vector.memset(ones_b[:], 1.0)

            # a_self / a_neigh as [64, H] columns
            av_sb = const.tile([F, H], f32)
            nc.sync.dma_start(av_sb[:], as_d.rearrange("h o -> o h"))
            an_sb = const.tile([F, H], f32)
            nc.sync.dma_start(an_sb[:], an_d.rearrange("h o -> o h"))

            # ---- X -> XT_ext [65, 2048] (f32, with ones row 64) -------
            x_sb = const.tile([P, NT * F], f32)
            nc.sync.dma_start(
                x_sb.rearrange("p (t f) -> p t f", f=F),
                X_d.rearrange("(t p) f -> p t f", p=P),
            )
            xTps = ps.tile([F, N], f32, tag="ps")
            for t in range(NT):
                nc.tensor.transpose(
                    xTps[:, t * P : (t + 1) * P],
                    x_sb[:, t * F : (t + 1) * F],
                    ident[:],
                )
            XT_ext = big.tile([F + 1, N], f32)
            nc.scalar.copy(XT_ext[0:F, :], xTps[:])
            nc.vector.memset(XT_ext[F : F + 1, :], 1.0)

            # ---- A^T via bf16-bitcast xbar transpose + perm-merge -----
            AT_sb = big.tile([P, NT * N], fp16)
            Vb = A_d.bitcast(bf16)  # [2048, 4096]
            for k in range(NT):
                ta = stream.tile([P, N], bf16, tag="tt")
                nc.sync.dma_start_transpose(
                    ta[:], Vb[:, 256 * k : 256 * k + 128]
                )
                tb = stream.tile([P, N], bf16, tag="tt")
                nc.sync.dma_start_transpose(
                    tb[:], Vb[:, 256 * k + 128 : 256 * k + 256]
                )
                psA = ps.tile([P, N], f32, tag="ps")
                for c in range(NCH):
                    sl = slice(c * C, (c + 1) * C)
                    nc.tensor.matmul(
                        psA[:, sl], perm_a[:], ta[:, sl], start=True, stop=False
                    )
                    nc.tensor.matmul(
                        psA[:, sl], perm_b[:], tb[:, sl], start=False, stop=True
                    )
                nc.scalar.copy(AT_sb[:, k * N : (k + 1) * N], psA[:])

            # ---- per-head pipeline ------------------------------------
            for h in range(H):
                W_ext = head.tile([F + 1, FH], f32)
                nc.scalar.dma_start(W_ext[0:F, :], W_d[h])
                nc.scalar.dma_start(W_ext[F : F + 1, :], b_d[h : h + 1, :])

                # featsT [o, n] = W[h]^T X^T  (true feats, no bias)
                psF = ps.tile([FH, N], f32, tag="ps")
                for c in range(NCH):
                    sl = slice(c * C, (c + 1) * C)
                    nc.tensor.matmul(
                        psF[:, sl], W_ext[0:F, :], XT_ext[0:F, sl],
                        start=True, stop=True,
                    )
                featsT = head.tile([FH, N], f32)
                nc.scalar.copy(featsT[:], psF[:])

                # s_self row -> g = exp(-0.8 * s_self)
                psR = ps.tile([1, N], f32, tag="ps")
                for c in range(NCH):
                    sl = slice(c * C, (c + 1) * C)
                    nc.tensor.matmul(
                        psR[:, sl], av_sb[:, h : h + 1], featsT[:, sl],
                        start=True, stop=True,
                    )
                g_row = head.tile([1, N], fp16)
                nc.scalar.activation(g_row[:], psR[:], act.Exp, scale=-0.8)

                # feats + b -> G  (G[:,64]=1), via [X|1] @ [W;b]
                psG = ps.tile([P, NT * FH], f32, tag="ps")
                for k in range(NT):
                    nc.tensor.matmul(
                        psG[:, k * FH : (k + 1) * FH],
                        XT_ext[:, k * P : (k + 1) * P],
                        W_ext[:],
                        start=True, stop=True,
                    )
                G_all = head.tile([P, NT * GW], fp16)
                G3 = G_all.rearrange("p (k w) -> p k w", w=GW)
                nc.scalar.copy(
                    G3[:, :, 0:FH], psG.rearrange("p (k f) -> p k f", f=FH)
                )
                nc.vector.memset(G3[:, :, FH : FH + 1], 1.0)

                # s_neigh grid -> e1, e2
                psNg = ps.tile([P, NT], f32, tag="ps")
                for k in range(NT):
                    nc.tensor.matmul(
                        psNg[:, k : k + 1],
                        featsT[:, k * P : (k + 1) * P],
                        an_sb[:, h : h + 1],
                        start=True, stop=True,
                    )
                e1g = head.tile([P, NT], f32)
                nc.scalar.activation(e1g[:], psNg[:], act.Exp, scale=1.0)
                e2g = head.tile([P, NT], f32)
                nc.scalar.activation(e2g[:], psNg[:], act.Exp, scale=0.2)

                # g broadcast [128, 2048] bf16
                psB = ps.tile([P, N], f32, tag="ps")
                for c in range(NCH):
                    sl = slice(c * C, (c + 1) * C)
                    nc.tensor.matmul(
                        psB[:, sl], ones_h[:], g_row[:, sl], start=True, stop=True
                    )
                g_bc = head.tile([P, N], fp16)
                nc.scalar.copy(g_bc[:], psB[:])

                # main loop over m-tiles
                agg = ps.tile([FH + 1, N], f32, tag="ps")
                for k in range(NT):
                    u_t = stream.tile([P, N], fp16, tag="u")
                    nc.vector.tensor_scalar(
                        u_t[:], g_bc[:],
                        e2g[:, k : k + 1], e1g[:, k : k + 1],
                        op.mult, op.max,
                    )
                    p_t = stream.tile([P, N], fp16, tag="p")
                    nc.vector.tensor_tensor(
                        p_t[:], u_t[:], AT_sb[:, k * N : (k + 1) * N], op.mult
                    )
                    for c in range(NCH):
                        sl = slice(c * C, (c + 1) * C)
                        nc.tensor.matmul(
                            agg[:, sl],
                            G_all[:, k * GW : k * GW + FH + 1],
                            p_t[:, sl],
                            start=(k == 0), stop=(k == NT - 1),
                        )

                # normalize + relu + store (transposed)
                lnr = head.tile([1, N], f32)
                nc.scalar.activation(lnr[:], agg[FH : FH + 1, :], act.Ln)
                rrow = head.tile([1, N], f32)
                nc.scalar.activation(rrow[:], lnr[:], act.Exp, scale=-1.0)
                for c in range(NCH):
                    sl = slice(c * C, (c + 1) * C)
                    psRc = ps.tile([FH + 1, C], f32, tag="ps")
                    nc.tensor.matmul(
                        psRc[:], ones_f[0:1, 0 : FH + 1], rrow[:, sl],
                        start=True, stop=True,
                    )
                    rc_sb = head.tile([FH + 1, C], f32)
                    nc.scalar.copy(rc_sb[:], psRc[:])
                    outf = outp.tile([FH + 1, C], f32)
                    nc.vector.scalar_tensor_tensor(
                        outf[:], agg[:, sl], 0.0, rc_sb[:], op.max, op.mult
                    )
                    nc.scalar.dma_start(OUT_d[h, :, sl], outf[0:FH, :])

    nc.compile()
    return nc


def _get_nc():
    if "nc" not in _CACHE:
        _CACHE["nc"] = _build()
    return _CACHE["nc"]


def make_in_maps(inputs):
    X = np.ascontiguousarray(inputs["X"], dtype=np.float32)
    A = np.ascontiguousarray(inputs["A"], dtype=np.float32)
    W = np.ascontiguousarray(inputs["W"], dtype=np.float32)
    b = np.ascontiguousarray(inputs["b"], dtype=np.float32)
    a_self = np.ascontiguousarray(inputs["a_self"], dtype=np.float32)
    a_neigh = np.ascontiguousarray(inputs["a_neigh"], dtype=np.float32)
    return [
        {
            "A": np.ascontiguousarray(A[i]),
            "X": np.ascontiguousarray(X[i]),
            "W": W,
            "b": b,
            "a_self": a_self,
            "a_neigh": a_neigh,
        }
        for i in range(B)
    ]


def run(inputs, trace=False):
    from concourse import bass_utils

    nc = _get_nc()
    in_maps = make_in_maps(inputs)
    res = bass_utils.run_bass_kernel_spmd(
        nc, in_maps, core_ids=list(range(B)), trace=trace
    )
    out = np.empty((B, N, H * FH), dtype=np.float32)
    for i in range(B):
        o = res.results[i]["OUT"]  # [H, FH, N]
        out[i] = o.transpose(2, 0, 1).reshape(N, H * FH)
    return out, res


def kernel(**inputs):
    out, _ = run(inputs, trace=False)
    return out


# revision 26
# speedup vs baseline: 21.6065x; 21.6065x over previous
"""Batch graph attention (GAT-style) Trainium2 kernel.

Problem: B=8, N=2048, F=64, FH=64, H=4.
  feats = X @ W[h]                         [B,H,N,FH]
  scores[n,m] = leaky_relu(s_self[n] + s_neigh[m], 0.2)
  P = softmax(scores + (1-A)*NEG_BIG, axis=m)
  out = relu(concat_h(P @ feats + b))

Sharding: batch b -> core b (8 cores, data parallel).

Per-core algorithm (all in "transposed" orientation so the PE can reduce
over the neighbor index m, which must sit on SBUF partitions):

  exp(leaky(x)) == max(e^x, e^{0.2x})  (slope<1), and each branch factors
  rank-1 over (n, m).  Dropping the per-column factor e^{s_self[n]}
  (softmax columns are scale invariant) leaves

      Phat[m,n] = A^T[m,n] * max(e1[m], e2[m] * g[n])

  with e1=exp(s_neigh), e2=exp(0.2*s_neigh), g=exp(-0.8*s_self).
  Aggregation + denominators come from one PE matmul stream per m-tile:

      acc[o,n] += G[m,o]^T Phat[m,n],   G = [feats + b | 1]

  and out[n, h*64+o] = relu(acc[o,n] / acc[64,n]) is produced transposed
  ([H,FH,N] per core) and untransposed on the host during unsharding.

  A^T is produced on-chip by bitcasting the fp32 A (values 0.0/1.0) to
  bf16 pairs [0x0000 | bf16(A)], xbar-DMA-transposing 128-column chunks
  (interleaved zero rows), and compacting odd partitions with two
  constant permutation matmuls on the PE.
"""

import numpy as np

B, N, F, FH, H = 8, 2048, 64, 64, 4
P = 128           # SBUF partitions
NT = N // P       # 16 m-tiles
C = 512           # matmul moving-operand chunk
NCH = N // C      # 4 chunks
GW = 66           # G row stride (64 feats + 1 ones + 1 pad)

_CACHE = {}

# tuning knobs (read at build time)
KNOBS = {
    "psm_bufs": 3,        # merge psum chunk buffers (1 bank each)
    "pst_bufs": 1,        # transient psum buffers
    "at_dve": 1,          # of every 4 AT merge copies, this many go to DVE
    "gps_stride": 4,      # every gps_stride-th op-II tile goes to GPSIMD (0=off)
    "tt_bufs": 8,         # xbar staging tile buffers
}


def _build():
    import concourse.bacc as bacc
    import concourse.tile as tile
    import concourse.mybir as mybir
    from concourse.mybir import AluOpType as op, ActivationFunctionType as act

    f32 = mybir.dt.float32
    bf16 = mybir.dt.bfloat16
    fp16 = mybir.dt.float16
    i32 = mybir.dt.int32

    nc = bacc.Bacc(
        "TRN2",
        target_bir_lowering=False,
        debug=False,
        enable_asserts=False,
        num_devices=8,
    )

    A_d = nc.dram_tensor("A", [N, N], f32, kind="ExternalInput").ap()
    X_d = nc.dram_tensor("X", [N, F], f32, kind="ExternalInput").ap()
    W_d = nc.dram_tensor("W", [H, F, FH], f32, kind="ExternalInput").ap()
    b_d = nc.dram_tensor("b", [H, FH], f32, kind="ExternalInput").ap()
    as_d = nc.dram_tensor("a_self", [H, FH], f32, kind="ExternalInput").ap()
    an_d = nc.dram_tensor("a_neigh", [H, FH], f32, kind="ExternalInput").ap()
    OUT_d = nc.dram_tensor("OUT", [H, FH, N], f32, kind="ExternalOutput").ap()

    with tile.TileContext(nc) as tc:
        with (
            tc.tile_pool(name="const", bufs=1) as const,
            tc.tile_pool(name="big", bufs=1) as big,
            tc.tile_pool(name="stream", bufs=3) as stream,
            tc.tile_pool(name="head", bufs=2) as head,
            tc.tile_pool(name="outp", bufs=3) as outp,
            # PSUM: merge chunks (1 bank) + agg halves (3x2 banks) + transients (1)
            tc.tile_pool(name="psm", bufs=3, space="PSUM") as psm,
            tc.tile_pool(name="psagg", bufs=2, space="PSUM") as psagg,
            tc.tile_pool(name="pst", bufs=1, space="PSUM") as pst,
        ):
            # ---- constants --------------------------------------------
            iota_i = const.tile([P, P], i32)
            nc.gpsimd.iota(iota_i[:], pattern=[[1, P]], base=0, channel_multiplier=0)
            pidx_i = const.tile([P, 1], i32)
            nc.gpsimd.iota(pidx_i[:], pattern=[[0, 1]], base=0, channel_multiplier=1)
            iota_f = const.tile([P, P], f32)
            nc.vector.tensor_copy(iota_f[:], iota_i[:])
            pidx_f = const.tile([P, 1], f32)
            nc.vector.tensor_copy(pidx_f[:], pidx_i[:])
            ident = const.tile([P, P], fp16)
            nc.vector.tensor_scalar(ident[:], iota_f[:], pidx_f[:], None, op.is_equal)
            pm1 = const.tile([P, 1], f32)
            nc.vector.tensor_scalar(pm1[:], pidx_f[:], 1.0, None, op.subtract)
            pp127 = const.tile([P, 1], f32)
            nc.vector.tensor_scalar(pp127[:], pidx_f[:], 127.0, None, op.add)
            # perm_a[p,q]=1 iff p==2q+1 ; perm_b[p,q]=1 iff p==2q-127
            perm_a = const.tile([P, P], bf16)
            nc.vector.tensor_scalar(
                perm_a[:], iota_f[:], 2.0, pm1[:], op.mult, op.is_equal
            )
            perm_b = const.tile([P, P], bf16)
            nc.vector.tensor_scalar(
                perm_b[:], iota_f[:], 2.0, pp127[:], op.mult, op.is_equal
            )

            # a_self / a_neigh as fp16 [64, H] columns (SWDGE cast DMA)
            av16 = const.tile([F, H], fp16)
            nc.gpsimd.dma_start(av16[:], as_d.rearrange("h o -> o h"))
            an16 = const.tile([F, H], fp16)
            nc.gpsimd.dma_start(an16[:], an_d.rearrange("h o -> o h"))

            # ---- X -> XT16 [65, 2048] (fp16, ones row 64) -------------
            x16 = const.tile([P, NT * F], fp16)
            nc.gpsimd.dma_start(
                x16.rearrange("p (t f) -> p t f", f=F),
                X_d.rearrange("(t p) f -> p t f", p=P),
            )
            XT16 = big.tile([F + 1, N], fp16)
            xTps = psagg.tile([F, N], fp16, tag="agg")
            for t in range(NT):
                nc.tensor.transpose(
                    xTps[:, t * P : (t + 1) * P],
                    x16[:, t * F : (t + 1) * F],
                    ident[:],
                )
            nc.scalar.copy(XT16[0:F, :], xTps[:])
            nc.vector.memset(XT16[F : F + 1, :], 1.0)

            # ---- A^T via bf16-bitcast xbar transpose + perm-merge -----
            AT_sb = big.tile([P, NT * N], fp16)
            Vb = A_d.bitcast(bf16)  # [2048, 4096]
            for k in range(NT):
                ta = stream.tile([P, N], bf16, tag="tt", bufs=KNOBS["tt_bufs"])
                nc.sync.dma_start_transpose(ta[:], Vb[:, 256 * k : 256 * k + 128])
                tb = stream.tile([P, N], bf16, tag="tt", bufs=KNOBS["tt_bufs"])
                nc.sync.dma_start_transpose(
                    tb[:], Vb[:, 256 * k + 128 : 256 * k + 256]
                )
                for c in range(NCH):
                    sl = slice(c * C, (c + 1) * C)
                    psA = psm.tile([P, C], f32, tag="m")
                    nc.tensor.matmul(
                        psA[:], perm_a[:], ta[:, sl], start=True, stop=False
                    )
                    nc.tensor.matmul(
                        psA[:], perm_b[:], tb[:, sl], start=False, stop=True
                    )
                    dst = AT_sb[:, k * N + c * C : k * N + (c + 1) * C]
                    if (k * NCH + c) % 4 < KNOBS["at_dve"]:
                        nc.vector.tensor_copy(dst, psA[:])
                    else:
                        nc.scalar.copy(dst, psA[:])

            # ---- per-head pipeline ------------------------------------
            for h in range(H):
                # [W[h]; b[h]] as fp16 [65, 64] (SWDGE cast DMA)
                W16 = head.tile([F + 1, FH], fp16, tag="W16", bufs=2)
                nc.gpsimd.dma_start(W16[0:F, :], W_d[h])
                nc.gpsimd.dma_start(W16[F : F + 1, :], b_d[h : h + 1, :])

                # featsT16 [o, n] = W^T X^T (true feats, no bias)
                featsT = head.tile([FH, N], fp16, tag="featsT", bufs=2)
                for c in range(NCH):
                    sl = slice(c * C, (c + 1) * C)
                    psF = pst.tile([FH, C], f32, tag="t")
                    nc.tensor.matmul(
                        psF[:], W16[0:F, :], XT16[0:F, sl],
                        start=True, stop=True,
                    )
                    nc.scalar.copy(featsT[:, sl], psF[:])

                # s_neigh grid -> e1, e2 ; s_self grid -> transposed row
                psNg = pst.tile([P, 2 * NT], f32, tag="t")
                for k in range(NT):
                    nc.tensor.matmul(
                        psNg[:, k : k + 1],
                        featsT[:, k * P : (k + 1) * P],
                        an16[:, h : h + 1],
                        start=True, stop=True,
                    )
                    nc.tensor.matmul(
                        psNg[:, NT + k : NT + k + 1],
                        featsT[:, k * P : (k + 1) * P],
                        av16[:, h : h + 1],
                        start=True, stop=True,
                    )
                e1g = head.tile([P, NT], f32, tag="e1g", bufs=2)
                nc.scalar.activation(e1g[:], psNg[:, 0:NT], act.Exp, scale=1.0)
                e2g = head.tile([P, NT], f32, tag="e2g", bufs=2)
                nc.scalar.activation(e2g[:], psNg[:, 0:NT], act.Exp, scale=0.2)
                ssg = head.tile([P, NT], fp16, tag="ssg", bufs=2)
                nc.scalar.copy(ssg[:], psNg[:, NT : 2 * NT])

                # g_row = exp(-0.8 * s_self) as [1, 2048] fp16
                g_row = head.tile([1, N], fp16, tag="g_row", bufs=2)
                for c in range(NCH):
                    psRow = pst.tile([1, C], fp16, tag="t")
                    for j in range(4):
                        kk = c * 4 + j
                        nc.tensor.transpose(
                            psRow[:, j * P : (j + 1) * P],
                            ssg[:, kk : kk + 1],
                            ident[:],
                        )
                    nc.scalar.activation(
                        g_row[:, c * C : (c + 1) * C], psRow[:], act.Exp, scale=-0.8
                    )
                g_bc = head.tile([P, N], fp16, tag="g_bc", bufs=2)
                nc.gpsimd.partition_broadcast(g_bc[:], g_row[:])

                # G = [feats + b | 1] fp16, via [X|1] @ [W;b]
                G_all = head.tile([P, NT * GW], fp16, tag="G_all", bufs=2)
                G3 = G_all.rearrange("p (k w) -> p k w", w=GW)
                for halfg in range(2):
                    psG = pst.tile([P, (NT // 2) * FH], f32, tag="t")
                    for j in range(NT // 2):
                        k = halfg * (NT // 2) + j
                        nc.tensor.matmul(
                            psG[:, j * FH : (j + 1) * FH],
                            XT16[:, k * P : (k + 1) * P],
                            W16[:],
                            start=True, stop=True,
                        )
                    nc.scalar.copy(
                        G3[:, halfg * (NT // 2) : (halfg + 1) * (NT // 2), 0:FH],
                        psG.rearrange("p (k f) -> p k f", f=FH),
                    )
                nc.vector.memset(G3[:, :, FH : FH + 1], 1.0)

                # agg kept as two 2-bank halves
                aggs = [
                    psagg.tile([FH + 1, N // 2], f32, tag="agg", name=f"agg{h}_{c}")
                    for c in range(2)
                ]
                for k in range(NT):
                    u_t = stream.tile([P, N], fp16, tag="u", bufs=3)
                    nc.vector.tensor_scalar(
                        u_t[:], g_bc[:],
                        e2g[:, k : k + 1], e1g[:, k : k + 1],
                        op.mult, op.max,
                    )
                    p_t = stream.tile([P, N], fp16, tag="p", bufs=6)
                    gs = KNOBS["gps_stride"]
                    eng = nc.gpsimd if (gs and k % gs == gs - 1) else nc.vector
                    eng.tensor_tensor(
                        p_t[:], u_t[:], AT_sb[:, k * N : (k + 1) * N], op.mult
                    )
                    for c in range(NCH):
                        sl = slice(c * C, (c + 1) * C)
                        nc.tensor.matmul(
                            aggs[c // 2][:, (c % 2) * C : (c % 2 + 1) * C],
                            G_all[:, k * GW : k * GW + FH + 1],
                            p_t[:, sl],
                            start=(k == 0), stop=(k == NT - 1),
                        )

                # normalize + relu + store (transposed)
                rrow = head.tile([1, N], f32, tag="rrow", bufs=2)
                for half in range(2):
                    lnr = head.tile([1, N // 2], f32, tag="lnr", bufs=2)
                    nc.scalar.activation(
                        lnr[:], aggs[half][FH : FH + 1, :], act.Ln
                    )
                    nc.scalar.activation(
                        rrow[:, half * (N // 2) : (half + 1) * (N // 2)],
                        lnr[:], act.Exp, scale=-1.0,
                    )
                rbc = head.tile([FH + 1, N], f32, tag="rbc", bufs=1)
                nc.gpsimd.partition_broadcast(rbc[:], rrow[:])
                for c in range(NCH):
                    sl = slice(c * C, (c + 1) * C)
                    outf = outp.tile([FH + 1, C], f32)
                    nc.vector.scalar_tensor_tensor(
                        outf[:],
                        aggs[c // 2][:, (c % 2) * C : (c % 2 + 1) * C],
                        0.0, rbc[:, sl], op.max, op.mult,
                    )
                    nc.scalar.dma_start(OUT_d[h, :, sl], outf[0:FH, :])

    nc.compile()
    return nc


def _get_nc():
    if "nc" not in _CACHE:
        _CACHE["nc"] = _build()
    return _CACHE["nc"]


def make_in_maps(inputs):
    X = np.ascontiguousarray(inputs["X"], dtype=np.float32)
    A = np.ascontiguousarray(inputs["A"], dtype=np.float32)
    W = np.ascontiguousarray(inputs["W"], dtype=np.float32)
    b = np.ascontiguousarray(inputs["b"], dtype=np.float32)
    a_self = np.ascontiguousarray(inputs["a_self"], dtype=np.float32)
    a_neigh = np.ascontiguousarray(inputs["a_neigh"], dtype=np.float32)
    return [
        {
            "A": np.ascontiguousarray(A[i]),
            "X": np.ascontiguousarray(X[i]),
            "W": W,
            "b": b,
            "a_self": a_self,
            "a_neigh": a_neigh,
        }
        for i in range(B)
    ]


def run(inputs, trace=False):
    from concourse import bass_utils

    nc = _get_nc()
    in_maps = make_in_maps(inputs)
    res = bass_utils.run_bass_kernel_spmd(
        nc, in_maps, core_ids=list(range(B)), trace=trace
    )
    out = np.empty((B, N, H * FH), dtype=np.float32)
    for i in range(B):
        o = res.results[i]["OUT"]  # [H, FH, N]
        out[i] = o.transpose(2, 0, 1).reshape(N, H * FH)
    return out, res


def kernel(**inputs):
    out, _ = run(inputs, trace=False)
    return out
